# revision 1
# baseline (speedup 1.0000x reference)
"""Correlation cost volume kernel for Trainium2 (8 NeuronCores, data-parallel over batch).

cost[b, i, h, x] = mean_c left[b,c,h,x] * right[b,c,h,x-i],  i in [0,48), zero for x < i.

Strategy per core (one batch element per core):
  For each h row (96) and x-chunk X0 (3 chunks M=128/128/64):
    PSUM G[a, j] = sum_c left[c, X0+a] * right_pad[c, X0+j]   (right_pad: 47 left zeros)
    band[a, k] = G[a, a+k] = cost[47-k, X0+a]  -> the 48-diagonal band
  Band extraction (the "shear") is a DMA with a diagonal access pattern
  (partition step = row+1), or a DRAM-roundtrip fallback.
  PE-transpose band -> [48, M]; assemble [48, 8h x 320] group tiles; one
  strided DMA per group writes the (i, h, x) output layout directly
  (disparity written in reverse order k = 47-i; host flips).
"""
import numpy as np

import concourse.bacc as bacc
import concourse.bass as bass
import concourse.mybir as mybir
import concourse.tile as tile
from concourse.ap import AP
from concourse.bass_utils import run_bass_kernel_spmd

import os
B, C, H, W = 8, 128, 96, 320
NG_LIMIT = int(os.environ.get("NG_LIMIT", "0"))
D = 48  # disparities
RPAD = 512 if os.environ.get("FP32R") else 384
HG = int(os.environ.get("HG", "8"))  # h rows per group
NG = H // HG  # 12 groups
CHUNKS = [(0, 128), (128, 128), (256, 64)]  # (X0, M)
HW = H * W

SHEAR_MODE = "dram"  # "diag" | "dram"

_cache = {}


def _build(shear_mode):
    nc = bacc.Bacc("TRN2", target_bir_lowering=False, debug=False, num_devices=8)
    left = nc.dram_tensor("left", [C, H * W], mybir.dt.float32, kind="ExternalInput").ap()
    right = nc.dram_tensor("right", [C, H * W], mybir.dt.float32, kind="ExternalInput").ap()
    ident_in = nc.dram_tensor("ident", [128, 128], mybir.dt.float32, kind="ExternalInput").ap()
    out = nc.dram_tensor("out", [D, H * W], mybir.dt.float32, kind="ExternalOutput").ap()
    if shear_mode == "dram":
        # quarter-block scratch, double-buffered across groups. AB chunks
        # (M=128) span all 4 quarters; chunk C (M=64) only quarters 0-1.
        scrAB = [
            [nc.dram_tensor(f"scrab_{par}_{q}", [32, HG * 2, 80], mybir.dt.float32).ap()
             for q in range(4)]
            for par in range(2)
        ]
        scrC = [
            [nc.dram_tensor(f"scrc_{par}_{q}", [32, HG, 80], mybir.dt.float32).ap()
             for q in range(2)]
            for par in range(2)
        ]

    with tile.TileContext(nc) as tc:
        with (
            tc.tile_pool(name="io", bufs=int(os.environ.get("IOBUFS", "2"))) as io_pool,
            tc.tile_pool(name="work", bufs=3) as work_pool,
            tc.tile_pool(name="bandp", bufs=int(os.environ.get("BANDBUFS", "3"))) as band_pool,
            tc.tile_pool(name="outp", bufs=3) as out_pool,
            tc.tile_pool(name="const", bufs=1) as const_pool,
            tc.tile_pool(name="ps", bufs=int(os.environ.get("PSBUFS", "4")), space="PSUM") as ps_pool,
            tc.tile_pool(name="ps2", bufs=3, space="PSUM") as ps2_pool,
        ):
            ident = const_pool.tile([128, 128], mybir.dt.float32)
            nc.sync.dma_start(out=ident[:, :], in_=ident_in[:, :])

            for g in range(NG_LIMIT or NG):
                h0 = g * HG
                l_t = io_pool.tile([C, HG * W], mybir.dt.float32, tag="lt")
                r_t = io_pool.tile([C, HG * RPAD], mybir.dt.float32, tag="rt")
                # zero the pad columns of r_t:  [0:47] and [W+47:RPAD] per h row
                if not os.environ.get("SKIP_MEMSET"):
                    nc.gpsimd.memset(
                        AP(r_t.tensor, r_t.offset, [[r_t.ap[0][0], C], [RPAD, HG], [1, D - 1]]),
                        0.0,
                    )
                    nc.gpsimd.memset(
                        AP(r_t.tensor, r_t.offset + W + D - 1,
                           [[r_t.ap[0][0], C], [RPAD, HG], [1, RPAD - W - D + 1]]),
                        0.0,
                    )
                NL = int(os.environ.get("NLOAD", "4"))
                HH = HG // NL
                for li in range(NL):
                    nc.scalar.dma_start(
                        out=l_t[:, li * HH * W : (li + 1) * HH * W],
                        in_=left[:, (h0 + li * HH) * W : (h0 + (li + 1) * HH) * W])
                # strided load of right into padded rows at offset 47
                if os.environ.get("SKIP_RSTRIDE"):
                    nc.sync.dma_start(out=r_t[:, : HG * W], in_=right[:, h0 * W : (h0 + HG) * W])
                else:
                    r_dst = AP(r_t.tensor, r_t.offset + (D - 1),
                               [[r_t.ap[0][0], C], [RPAD, HG], [1, W]])
                    for li in range(NL):
                        r_dsti = AP(r_t.tensor, r_t.offset + li * HH * RPAD + (D - 1),
                                    [[r_t.ap[0][0], C], [RPAD, HH], [1, W]])
                        r_eng = nc.scalar if os.environ.get("R_ON_ACT") else nc.sync
                        r_eng.dma_start(
                            out=r_dsti,
                            in_=right[:, (h0 + li * HH) * W : (h0 + (li + 1) * HH) * W])

                bandT_g = out_pool.tile([D, HG * W], mybir.dt.float32, tag="btg")

                if shear_mode == "dram":
                    rect_g = band_pool.tile([128, HG * 3 * 176], mybir.dt.float32, tag="rectg")

                for hl in range(HG):
                    for ci, (X0, M) in enumerate(CHUNKS):
                        NW = M + D  # 176 or 112: useful rhs window width
                        NMM = 256 if os.environ.get("FP32R") else NW
                        g_ps = ps_pool.tile([M, NMM], mybir.dt.float32, tag="gps")
                        if not os.environ.get("FP32R"):
                            nc.tensor.matmul(
                                g_ps[:, :NMM],
                                l_t[:, hl * W + X0 : hl * W + X0 + M],
                                r_t[:, hl * RPAD + X0 : hl * RPAD + X0 + NMM],
                                start=True, stop=True,
                            )
                        else:
                            nc.tensor.matmul(
                                g_ps[:, :NMM],
                                l_t[:, hl * W + X0 : hl * W + X0 + M].bitcast(mybir.dt.float32r),
                                r_t[:, hl * RPAD + X0 : hl * RPAD + X0 + NMM].bitcast(mybir.dt.float32r),
                                start=True, stop=True,
                            )
                        if shear_mode == "diag":
                            band = band_pool.tile([128, D], mybir.dt.float32, tag="band")
                            rect = work_pool.tile([M, 176], mybir.dt.float32, tag="rect")
                            nc.vector.tensor_scalar_mul(rect[:, :NW], g_ps[:, :NW], 1.0 / C)
                            s = rect.ap[0][0]
                            assert s == 176
                            diag = AP(rect.tensor, rect.offset, [[s + 1, M], [1, D]])
                            nc.sync.dma_start(out=band[:M, :], in_=diag)
                        else:
                            roff = (ci * HG + hl) * 176
                            nc.vector.tensor_scalar_mul(
                                rect_g[:M, roff : roff + NW], g_ps[:, :NW], 1.0 / C
                            )
                        if shear_mode == "diag":
                            bT_ps = ps2_pool.tile([D, 128], mybir.dt.float32, tag="btps")
                            nc.tensor.transpose(bT_ps[:, :M], band[:M, :], ident[:M, :M])
                            nc.vector.tensor_copy(
                                bandT_g[:, hl * W + X0 : hl * W + X0 + M], bT_ps[:, :M]
                            )

                if shear_mode == "dram":
                    NCH = HG * 3  # 24 chunk slots
                    # dump 4 quarter-block DMAs: rows [32q,32q+32), cols [32q, 32q+80) of each slot
                    rect_v3 = rect_g[:, :].rearrange("p (s w) -> p s w", s=NCH)
                    band_g = band_pool.tile([128, NCH * D], mybir.dt.float32, tag="bandg")
                    band_gv3 = band_g[:, :].rearrange("p (s k) -> p s k", s=NCH)
                    par = g % 2
                    NAB = 2 * HG  # slots 0..15 are chunks A,B (ci-major); 16..23 chunk C
                    for q in range(4):
                        eng = nc.sync if (os.environ.get("NO_DUMP_SPLIT") is None and q % 2) else nc.scalar
                        eng.dma_start(
                            out=scrAB[par][q],
                            in_=rect_v3[32 * q : 32 * q + 32, 0:NAB, 32 * q : 32 * q + 80],
                        )
                    for q in range(2):
                        eng = nc.sync if (os.environ.get("NO_DUMP_SPLIT") is None and q % 2) else nc.scalar
                        eng.dma_start(
                            out=scrC[par][q],
                            in_=rect_v3[32 * q : 32 * q + 32, NAB:NCH, 32 * q : 32 * q + 80],
                        )
                    for q in range(4):
                        scr = scrAB[par][q]
                        srcq = AP(scr.tensor, scr.offset,
                                  [[NAB * 80 + 1, 32], [80, NAB], [1, D]])
                        nc.sync.dma_start(
                            out=band_gv3[32 * q : 32 * q + 32, 0:NAB, :], in_=srcq)
                    for q in range(2):
                        scr = scrC[par][q]
                        srcq = AP(scr.tensor, scr.offset,
                                  [[HG * 80 + 1, 32], [80, HG], [1, D]])
                        nc.sync.dma_start(
                            out=band_gv3[32 * q : 32 * q + 32, NAB:NCH, :], in_=srcq)
                    for hl in range(HG):
                        for ci, (X0, M) in enumerate(CHUNKS):
                            coff = (ci * HG + hl) * D
                            bT_ps = ps2_pool.tile([D, 128], mybir.dt.float32, tag="btps")
                            nc.tensor.transpose(
                                bT_ps[:, :M], band_g[:M, coff : coff + D], ident[:M, :M]
                            )
                            nc.scalar.copy(
                                bandT_g[:, hl * W + X0 : hl * W + X0 + M], bT_ps[:, :M]
                            )

                # out DMA: dest addr(k, hl, x) = (47-k)*HW + (h0+hl)*W + x
                # reversed k handled by host flip: write k rows at (k)*HW,
                # i.e. device writes dis-reversed volume rev[k] = cost[47-k].
                if os.environ.get("SKIP_OUTAP"):
                    nc.sync.dma_start(out=out[:, h0 * HG * 0 : HG * W], in_=bandT_g[:, :])
                else:
                    NO = int(os.environ.get("NOUT", "2"))
                    HO = HG // NO
                    for oi in range(NO):
                        dst = AP(out.tensor, out.offset + (h0 + oi * HO) * W,
                                 [[HW, D], [W, HO], [1, W]])
                        nc.sync.dma_start(out=dst, in_=bandT_g[:, oi * HO * W : (oi + 1) * HO * W])
    nc.compile()
    return nc


def _get_nc(shear_mode):
    if shear_mode not in _cache:
        _cache[shear_mode] = _build(shear_mode)
    return _cache[shear_mode]


def kernel(left_feature, right_feature):
    left_feature = np.asarray(left_feature, dtype=np.float32)
    right_feature = np.asarray(right_feature, dtype=np.float32)
    b, c, h, w = left_feature.shape
    assert (b, c, h, w) == (B, C, H, W)
    nc = _get_nc(SHEAR_MODE)
    ident = np.eye(128, dtype=np.float32)
    in_maps = []
    for i in range(B):
        in_maps.append({
            "left": np.ascontiguousarray(left_feature[i].reshape(C, H * W)),
            "right": np.ascontiguousarray(right_feature[i].reshape(C, H * W)),
            "ident": ident,
        })
    trace = bool(os.environ.get("KERNEL_TRACE"))
    res = run_bass_kernel_spmd(nc, in_maps, core_ids=list(range(B)), trace=trace)
    if trace:
        print("HW exec time:", res.exec_time_ns, "ns")
        print("mean exec:", res.mean_exec_time_ns, "max core:", res.max_exec_time_core_id)
        if res.instructions_and_trace:
            print("trace path:", res.instructions_and_trace[1])
        if res.profile_json:
            print("profile json:", res.profile_json)
    outs = []
    for i in range(B):
        rev = res.results[i]["out"].reshape(D, H, W)
        outs.append(rev[::-1])  # device wrote k = 47 - i
    return np.stack(outs, axis=0).astype(np.float32)


if __name__ == "__main__":
    rng = np.random.default_rng(0)
    lf = rng.standard_normal((B, C, H, W), dtype=np.float32)
    rf = rng.standard_normal((B, C, H, W), dtype=np.float32)
    got = kernel(lf, rf)
    # quick reference for b=0, a few spots
    for (bb, i, hh, xx) in [(0, 0, 0, 0), (0, 5, 10, 100), (1, 47, 95, 319), (2, 47, 3, 10)]:
        if xx >= i:
            want = float(np.dot(lf[bb, :, hh, xx], rf[bb, :, hh, xx - i]) / C)
        else:
            want = 0.0
        print((bb, i, hh, xx), "got", got[bb, i, hh, xx], "want", want)



# revision 4
# speedup vs baseline: 1.8485x; 1.8485x over previous
"""Correlation cost volume kernel for Trainium2 (8 NeuronCores, data-parallel over batch).

cost[b, i, h, x] = mean_c left[b,c,h,x] * right[b,c,h,x-i],  i in [0,48), zero for x < i.

Per core (one batch element), all data bf16 (host converts; left pre-scaled by 1/C):
  For each group of HG=8 h rows:
    l_t [C, 8*320], r_t [C, 8*368] (47 left-pad zeros + data + 1 zero col) in SBUF.
    Per h row, 3 matmuls G[a, j] = sum_c l[c, X0+a] r_pad[c, X0+j] into a psum
    bank slot (A[128x176] B[128x176] C[64x112] packed in one 512-col fp32 bank).
    Engine eviction (DVE/Act alternating, 2h per op) -> rect SBUF bf16
    (per-h 464-col slots). One plain dump rect -> scr DRAM; three diagonal
    readbacks (DRAM flat stride 3713 = row+1) -> band SBUF [128, 8*144]
    (band[a, h, ci*48+k] = G[a, ci-chunk, a+k] = cost[i=47-k, x=X0+a]).
    PE transposes (bf16): AB merged [128,96]->[96,128], C [64,48]->[48,64]
    into psum bf16 [96, 384] per 2h; engine copy -> outg [96, 8*192];
    3 strided DMAs write the (k, h, x) output (disparity reversed; host flips).
"""
import numpy as np
import ml_dtypes

import concourse.bacc as bacc
import concourse.mybir as mybir
import concourse.tile as tile
from concourse.ap import AP
from concourse.bass_utils import run_bass_kernel_spmd

B, C, H, W = 8, 128, 96, 320
D = 48
HG = 8          # h rows per group
NG = H // HG    # 12 groups
RPAD = W + D    # 368: 47 left zeros, W data, 1 right zero
CHUNKS = [(0, 128, 0), (128, 128, 176), (256, 64, 352)]  # (X0, M, gcol)
SLOT = 464      # rect cols per h row (176+176+112)
HW = H * W

SHEAR_MODE = "dram"  # kept for test.py compat
_cache = {}


def _build(_mode="dram"):
    nc = bacc.Bacc("TRN2", target_bir_lowering=False, debug=False, num_devices=8)
    left = nc.dram_tensor("left", [C, HW], mybir.dt.bfloat16, kind="ExternalInput").ap()
    right = nc.dram_tensor("right", [C, HW], mybir.dt.bfloat16, kind="ExternalInput").ap()
    ident_in = nc.dram_tensor("ident", [128, 128], mybir.dt.bfloat16, kind="ExternalInput").ap()
    out = nc.dram_tensor("out", [D, HW], mybir.dt.bfloat16, kind="ExternalOutput").ap()
    scr = [nc.dram_tensor(f"scr_{p}", [C, HG * SLOT], mybir.dt.bfloat16).ap() for p in range(2)]

    with tile.TileContext(nc) as tc:
        with (
            tc.tile_pool(name="io", bufs=2) as io_pool,
            tc.tile_pool(name="rectp", bufs=2) as rect_pool,
            tc.tile_pool(name="bandp", bufs=2) as band_pool,
            tc.tile_pool(name="outp", bufs=2) as outg_pool,
            tc.tile_pool(name="const", bufs=1) as const_pool,
            tc.tile_pool(name="gps", bufs=3, space="PSUM") as g_pool,
            tc.tile_pool(name="bts", bufs=2, space="PSUM") as bt_pool,
        ):
            ident = const_pool.tile([128, 128], mybir.dt.bfloat16)
            nc.sync.dma_start(out=ident[:, :], in_=ident_in[:, :])

            for g in range(NG):
                h0 = g * HG
                l_t = io_pool.tile([C, HG * W], mybir.dt.bfloat16, tag="lt")
                r_t = io_pool.tile([C, HG * RPAD], mybir.dt.bfloat16, tag="rt")
                rtp = r_t.ap[0][0]
                # zero pads: cols [0:47] and col 367 of each h row
                nc.gpsimd.memset(
                    AP(r_t.tensor, r_t.offset, [[rtp, C], [RPAD, HG], [1, D - 1]]), 0.0)
                nc.gpsimd.memset(
                    AP(r_t.tensor, r_t.offset + RPAD - 1, [[rtp, C], [RPAD, HG], [1, 1]]), 0.0)
                nc.sync.dma_start(out=l_t[:, :], in_=left[:, h0 * W : (h0 + HG) * W])
                r_dst = AP(r_t.tensor, r_t.offset + (D - 1), [[rtp, C], [RPAD, HG], [1, W]])
                nc.scalar.dma_start(out=r_dst, in_=right[:, h0 * W : (h0 + HG) * W])

                rect = rect_pool.tile([C, HG * SLOT], mybir.dt.bfloat16, tag="rect")
                band = band_pool.tile([C, HG * 3 * D], mybir.dt.bfloat16, tag="band")
                outg = outg_pool.tile([96, HG * 192], mybir.dt.bfloat16, tag="outg")
                rp = rect.ap[0][0]
                bp = band.ap[0][0]

                for p in range(4):  # 2h units
                    gt = g_pool.tile([128, 1024], mybir.dt.float32, tag="g")
                    gp = gt.ap[0][0]
                    for e in range(2):
                        hl = 2 * p + e
                        for X0, M, gcol in CHUNKS:
                            NW = M + D
                            nc.tensor.matmul(
                                gt[:M, 512 * e + gcol : 512 * e + gcol + NW],
                                l_t[:, hl * W + X0 : hl * W + X0 + M],
                                r_t[:, hl * RPAD + X0 : hl * RPAD + X0 + NW],
                                start=True, stop=True,
                            )
                    eng = nc.vector if p % 2 == 0 else nc.scalar
                    ev = eng.tensor_copy if p % 2 == 0 else eng.copy
                    ev(
                        AP(rect.tensor, rect.offset + 2 * p * SLOT,
                           [[rp, 128], [SLOT, 2], [1, 352]]),
                        AP(gt.tensor, gt.offset, [[gp, 128], [512, 2], [1, 352]]),
                    )
                    ev(
                        AP(rect.tensor, rect.offset + 2 * p * SLOT + 352,
                           [[rp, 64], [SLOT, 2], [1, 112]]),
                        AP(gt.tensor, gt.offset + 352, [[gp, 64], [512, 2], [1, 112]]),
                    )

                scr_g = scr[g % 2]
                nc.scalar.dma_start(out=scr_g[:, :], in_=rect[:, :])
                for ci, (X0, M, gcol) in enumerate(CHUNKS):
                    src = AP(scr_g.tensor, scr_g.offset + gcol,
                             [[HG * SLOT + 1, M], [SLOT, HG], [1, D]])
                    dst = AP(band.tensor, band.offset + ci * D,
                             [[bp, M], [3 * D, HG], [1, D]])
                    rb_eng = nc.sync if ci == 0 else nc.scalar
                    rb_eng.dma_start(out=dst, in_=src)

                for p in range(4):
                    bt = bt_pool.tile([96, 384], mybir.dt.bfloat16, tag="bt")
                    for e in range(2):
                        hl = 2 * p + e
                        nc.tensor.transpose(
                            bt[0:96, 192 * e : 192 * e + 128],
                            band[:, hl * 3 * D : hl * 3 * D + 96],
                            ident[:, :],
                        )
                        nc.tensor.transpose(
                            bt[0:48, 192 * e + 128 : 192 * e + 192],
                            band[0:64, hl * 3 * D + 96 : hl * 3 * D + 144],
                            ident[0:64, 0:64],
                        )
                    ceng = nc.vector.tensor_copy if p % 2 else nc.scalar.copy
                    ceng(outg[:, p * 384 : (p + 1) * 384], bt[:, :])

                # out DMAs: rev volume rev[k] = cost[47-k]; host flips.
                ogp = outg.ap[0][0]
                for part, coff, xoff, MW in ((0, 0, 0, 128), (48, 0, 128, 128), (0, 128, 256, 64)):
                    src = AP(outg.tensor, outg.offset + part * ogp + coff,
                             [[ogp, D], [192, HG], [1, MW]])
                    dst = AP(out.tensor, out.offset + h0 * W + xoff,
                             [[HW, D], [W, HG], [1, MW]])
                    nc.sync.dma_start(out=dst, in_=src)
    nc.compile()
    return nc


def _get_nc(_mode="dram"):
    if _mode not in _cache:
        _cache[_mode] = _build(_mode)
    return _cache[_mode]


def kernel(left_feature, right_feature):
    import os
    left_feature = np.asarray(left_feature, dtype=np.float32)
    right_feature = np.asarray(right_feature, dtype=np.float32)
    b, c, h, w = left_feature.shape
    assert (b, c, h, w) == (B, C, H, W)
    nc = _get_nc()
    ident = np.eye(128, dtype=np.float32).astype(ml_dtypes.bfloat16)
    lf = (left_feature * (1.0 / C)).astype(ml_dtypes.bfloat16)
    rf = right_feature.astype(ml_dtypes.bfloat16)
    in_maps = []
    for i in range(B):
        in_maps.append({
            "left": np.ascontiguousarray(lf[i].reshape(C, HW)),
            "right": np.ascontiguousarray(rf[i].reshape(C, HW)),
            "ident": ident,
        })
    trace = bool(os.environ.get("KERNEL_TRACE"))
    res = run_bass_kernel_spmd(nc, in_maps, core_ids=list(range(B)), trace=trace)
    if trace:
        print("HW exec time:", res.exec_time_ns, "ns")
    outs = []
    for i in range(B):
        rev = res.results[i]["out"].astype(np.float32).reshape(D, H, W)
        outs.append(rev[::-1])  # device wrote k = 47 - i
    return np.stack(outs, axis=0)


if __name__ == "__main__":
    rng = np.random.default_rng(0)
    lf = rng.standard_normal((B, C, H, W), dtype=np.float32)
    rf = rng.standard_normal((B, C, H, W), dtype=np.float32)
    got = kernel(lf, rf)
    for (bb, i, hh, xx) in [(0, 0, 0, 0), (0, 5, 10, 100), (1, 47, 95, 319), (2, 47, 3, 10), (3, 20, 50, 10)]:
        want = float(np.dot(lf[bb, :, hh, xx], rf[bb, :, hh, xx - i]) / C) if xx >= i else 0.0
        print((bb, i, hh, xx), "got", got[bb, i, hh, xx], "want", want)


# revision 5
# speedup vs baseline: 2.0348x; 1.1008x over previous
"""Correlation cost volume kernel for Trainium2 (8 NeuronCores, data-parallel over batch).

cost[b, i, h, x] = mean_c left[b,c,h,x] * right[b,c,h,x-i],  i in [0,48), zero for x < i.

Per core (one batch element), all data bf16 (host converts; left pre-scaled by 1/C):
  For each group of HG=8 h rows:
    l_t [C, 8*320], r_t [C, 8*368] (47 left-pad zeros + data + 1 zero col) in SBUF.
    Per h row, 3 matmuls G[a, j] = sum_c l[c, X0+a] r_pad[c, X0+j] into a psum
    bank slot (A[128x176] B[128x176] C[64x112] packed in one 512-col fp32 bank).
    Engine eviction (DVE/Act alternating, 2h per op) -> rect SBUF bf16
    (per-h 464-col slots). One plain dump rect -> scr DRAM; three diagonal
    readbacks (DRAM flat stride 3713 = row+1) -> band SBUF [128, 8*144]
    (band[a, h, ci*48+k] = G[a, ci-chunk, a+k] = cost[i=47-k, x=X0+a]).
    PE transposes (bf16): AB merged [128,96]->[96,128], C [64,48]->[48,64]
    into psum bf16 [96, 384] per 2h; engine copy -> outg [96, 8*192];
    3 strided DMAs write the (k, h, x) output (disparity reversed; host flips).
"""
import numpy as np
import ml_dtypes

import concourse.bacc as bacc
import concourse.mybir as mybir
import concourse.tile as tile
from concourse.ap import AP
from concourse.bass_utils import run_bass_kernel_spmd

B, C, H, W = 8, 128, 96, 320
D = 48
HG = 8          # h rows per group
NG = H // HG    # 12 groups
RPAD = W + D    # 368: 47 left zeros, W data, 1 right zero
CHUNKS = [(0, 128, 0), (128, 128, 176), (256, 64, 352)]  # (X0, M, gcol)
SLOT = 464      # rect cols per h row (176+176+112)
HW = H * W

SHEAR_MODE = "dram"  # kept for test.py compat
_cache = {}


def _build(_mode="dram"):
    nc = bacc.Bacc("TRN2", target_bir_lowering=False, debug=False, num_devices=8)
    left = nc.dram_tensor("left", [C, HW], mybir.dt.bfloat16, kind="ExternalInput").ap()
    right = nc.dram_tensor("right", [C, HW], mybir.dt.bfloat16, kind="ExternalInput").ap()
    ident_in = nc.dram_tensor("ident", [128, 128], mybir.dt.bfloat16, kind="ExternalInput").ap()
    out = nc.dram_tensor("out", [D, HW], mybir.dt.bfloat16, kind="ExternalOutput").ap()
    scr = [nc.dram_tensor(f"scr_{p}", [C, HG * SLOT], mybir.dt.bfloat16).ap() for p in range(3)]

    with tile.TileContext(nc) as tc:
        with (
            tc.tile_pool(name="io", bufs=3) as io_pool,
            tc.tile_pool(name="rectp", bufs=2) as rect_pool,
            tc.tile_pool(name="bandp", bufs=3) as band_pool,
            tc.tile_pool(name="outp", bufs=3) as outg_pool,
            tc.tile_pool(name="const", bufs=1) as const_pool,
            tc.tile_pool(name="gps", bufs=3, space="PSUM") as g_pool,
            tc.tile_pool(name="bts", bufs=2, space="PSUM") as bt_pool,
        ):
            ident = const_pool.tile([128, 128], mybir.dt.bfloat16)
            nc.sync.dma_start(out=ident[:, :], in_=ident_in[:, :])

            for g in range(NG):
                h0 = g * HG
                l_t = io_pool.tile([C, HG * W], mybir.dt.bfloat16, tag="lt")
                r_t = io_pool.tile([C, HG * RPAD], mybir.dt.bfloat16, tag="rt")
                rtp = r_t.ap[0][0]
                # zero pads: cols [0:47] and col 367 of each h row
                nc.gpsimd.memset(
                    AP(r_t.tensor, r_t.offset, [[rtp, C], [RPAD, HG], [1, D - 1]]), 0.0)
                nc.gpsimd.memset(
                    AP(r_t.tensor, r_t.offset + RPAD - 1, [[rtp, C], [RPAD, HG], [1, 1]]), 0.0)
                nc.sync.dma_start(out=l_t[:, :], in_=left[:, h0 * W : (h0 + HG) * W])
                r_dst = AP(r_t.tensor, r_t.offset + (D - 1), [[rtp, C], [RPAD, HG], [1, W]])
                nc.scalar.dma_start(out=r_dst, in_=right[:, h0 * W : (h0 + HG) * W])

                rect = rect_pool.tile([C, HG * SLOT], mybir.dt.bfloat16, tag="rect")
                band = band_pool.tile([C, HG * 3 * D], mybir.dt.bfloat16, tag="band")
                outg = outg_pool.tile([96, HG * 192], mybir.dt.bfloat16, tag="outg")
                rp = rect.ap[0][0]
                bp = band.ap[0][0]

                for p in range(4):  # 2h units
                    gt = g_pool.tile([128, 1024], mybir.dt.float32, tag="g")
                    gp = gt.ap[0][0]
                    for e in range(2):
                        hl = 2 * p + e
                        for X0, M, gcol in CHUNKS:
                            NW = M + D
                            nc.tensor.matmul(
                                gt[:M, 512 * e + gcol : 512 * e + gcol + NW],
                                l_t[:, hl * W + X0 : hl * W + X0 + M],
                                r_t[:, hl * RPAD + X0 : hl * RPAD + X0 + NW],
                                start=True, stop=True,
                            )
                    eng = nc.vector if p % 2 == 0 else nc.scalar
                    ev = eng.tensor_copy if p % 2 == 0 else eng.copy
                    ev(
                        AP(rect.tensor, rect.offset + 2 * p * SLOT,
                           [[rp, 128], [SLOT, 2], [1, 352]]),
                        AP(gt.tensor, gt.offset, [[gp, 128], [512, 2], [1, 352]]),
                    )
                    ev(
                        AP(rect.tensor, rect.offset + 2 * p * SLOT + 352,
                           [[rp, 64], [SLOT, 2], [1, 112]]),
                        AP(gt.tensor, gt.offset + 352, [[gp, 64], [512, 2], [1, 112]]),
                    )

                scr_g = scr[g % 3]
                nc.gpsimd.dma_start(out=scr_g[:, :], in_=rect[:, :])
                for ci, (X0, M, gcol) in enumerate(CHUNKS):
                    src = AP(scr_g.tensor, scr_g.offset + gcol,
                             [[HG * SLOT + 1, M], [SLOT, HG], [1, D]])
                    dst = AP(band.tensor, band.offset + ci * D,
                             [[bp, M], [3 * D, HG], [1, D]])
                    rb_eng = nc.gpsimd if ci != 1 else nc.scalar
                    rb_eng.dma_start(out=dst, in_=src)

                for p in range(4):
                    bt = bt_pool.tile([96, 384], mybir.dt.bfloat16, tag="bt")
                    for e in range(2):
                        hl = 2 * p + e
                        nc.tensor.transpose(
                            bt[0:96, 192 * e : 192 * e + 128],
                            band[:, hl * 3 * D : hl * 3 * D + 96],
                            ident[:, :],
                        )
                        nc.tensor.transpose(
                            bt[0:48, 192 * e + 128 : 192 * e + 192],
                            band[0:64, hl * 3 * D + 96 : hl * 3 * D + 144],
                            ident[0:64, 0:64],
                        )
                    ceng = nc.vector.tensor_copy if p % 2 else nc.scalar.copy
                    ceng(outg[:, p * 384 : (p + 1) * 384], bt[:, :])

                # out DMAs: rev volume rev[k] = cost[47-k]; host flips.
                ogp = outg.ap[0][0]
                for part, coff, xoff, MW in ((0, 0, 0, 128), (48, 0, 128, 128), (0, 128, 256, 64)):
                    src = AP(outg.tensor, outg.offset + part * ogp + coff,
                             [[ogp, D], [192, HG], [1, MW]])
                    dst = AP(out.tensor, out.offset + h0 * W + xoff,
                             [[HW, D], [W, HG], [1, MW]])
                    nc.sync.dma_start(out=dst, in_=src)
    nc.compile()
    return nc


def _get_nc(_mode="dram"):
    if _mode not in _cache:
        _cache[_mode] = _build(_mode)
    return _cache[_mode]


def kernel(left_feature, right_feature):
    import os
    left_feature = np.asarray(left_feature, dtype=np.float32)
    right_feature = np.asarray(right_feature, dtype=np.float32)
    b, c, h, w = left_feature.shape
    assert (b, c, h, w) == (B, C, H, W)
    nc = _get_nc()
    ident = np.eye(128, dtype=np.float32).astype(ml_dtypes.bfloat16)
    lf = (left_feature * (1.0 / C)).astype(ml_dtypes.bfloat16)
    rf = right_feature.astype(ml_dtypes.bfloat16)
    in_maps = []
    for i in range(B):
        in_maps.append({
            "left": np.ascontiguousarray(lf[i].reshape(C, HW)),
            "right": np.ascontiguousarray(rf[i].reshape(C, HW)),
            "ident": ident,
        })
    trace = bool(os.environ.get("KERNEL_TRACE"))
    res = run_bass_kernel_spmd(nc, in_maps, core_ids=list(range(B)), trace=trace)
    if trace:
        print("HW exec time:", res.exec_time_ns, "ns")
    outs = []
    for i in range(B):
        rev = res.results[i]["out"].astype(np.float32).reshape(D, H, W)
        outs.append(rev[::-1])  # device wrote k = 47 - i
    return np.stack(outs, axis=0)


if __name__ == "__main__":
    rng = np.random.default_rng(0)
    lf = rng.standard_normal((B, C, H, W), dtype=np.float32)
    rf = rng.standard_normal((B, C, H, W), dtype=np.float32)
    got = kernel(lf, rf)
    for (bb, i, hh, xx) in [(0, 0, 0, 0), (0, 5, 10, 100), (1, 47, 95, 319), (2, 47, 3, 10), (3, 20, 50, 10)]:
        want = float(np.dot(lf[bb, :, hh, xx], rf[bb, :, hh, xx - i]) / C) if xx >= i else 0.0
        print((bb, i, hh, xx), "got", got[bb, i, hh, xx], "want", want)
